# revision 34
# baseline (speedup 1.0000x reference)
"""VQ codebook-lookup kernel for Trainium2 (8 NeuronCores, data-parallel over batch).

e[b,t,:] = dictionary[argmin_n ||ze[b,t,:] - dictionary[n,:]||^2]

Per core: 4 batches x 2048 rows = 8192, tiled 64 x 128 rows.
score(t,n) = 2*ze.c_n - |c_n|^2; argmax_n score == argmin_n d2.

Host splits ze and 2*dict^T into bf16 hi/lo pairs (~18-bit operands),
pre-transposed so the device does no PE transposes. scores = zh.dh + zh.dl +
zl.dh accumulate in fp32 PSUM (2-bank [128,1024] tensors, 4-deep ring) on top
of a -|c|^2 bias row: tiles 0-3 get the bias from a K=3 bf16 norm-matmul
(start=True); later tiles get it from Act half-bank preloads (start=False
accumulation), which hides the bias add entirely. 12 bf16 matmuls per tile is
the whole PE load; dummy warmup matmuls before the stream bring the PE clock
to full speed so every real matmul runs at 2.4 GHz.

Act copies scores PSUM->SBUF (half-bank granularity, interleaved with the
next ring slot's bias preloads); DVE max + drain + max_index give the argmax
(drain: the DVE pipeline does not interlock the m8 RAW dependency). Indices
are re-laid out per 8-tile chunk into the 16-partition-wrapped, group-
replicated table dma_gather requires - on Pool SWDGE for chunks 0-6
(overlapped with compute), on SP HWDGE for the final chunk's tail. The final
gather/store is split 2x512 rows to pipeline DMA transfers; the last stores
issue from Act to stay off SP's in-order tail.
"""
import sys
if '/opt/trn_rl_repo' not in sys.path:
    sys.path.insert(0, '/opt/trn_rl_repo')

import numpy as np
import ml_dtypes
from contextlib import ExitStack

import concourse.bass as bass
import concourse.bacc as bacc
import concourse.mybir as mybir
from concourse.bass_utils import run_bass_kernel_spmd

B, T, D, N = 32, 2048, 256, 1024
CORES = 8
ROWS = (B // CORES) * T          # 8192 rows per core
NTILES = ROWS // 128             # 64
CHUNK = 8                        # tiles per gather chunk
REPLICATE = True
NZBUF = 8                        # zhl pair-load prefetch depth (16 tiles)

f32 = mybir.dt.float32
bf16 = mybir.dt.bfloat16
u16 = mybir.dt.uint16
i16 = mybir.dt.int16

_CACHE = {}

TERMS = [(0, 0), (1, 0), (0, 1)]  # (z plane, d plane): hh, lh, hl — dl last
                                  # so the split dict-lo load can arrive late


def build(ntiles=NTILES):
    nchunk = ntiles // CHUNK
    ndma_b = 8 + (7 if REPLICATE else 0)
    # per-chunk relayout into offset-0 ping-pong buffers (the only layout the
    # hw gather path tolerates). Chunks 0..nchunk-2 relayout on Pool SWDGE;
    # the final chunk's relayout runs on SP HWDGE at the tail.
    pool_chunks = list(range(nchunk - 1))
    sp_chunks = [nchunk - 1] if nchunk else []
    rel_sem = {}
    rel_after = {}
    rel_base = {}
    for group, name in ((pool_chunks, "relp"), (sp_chunks, "rels")):
        for j, g in enumerate(group):
            rel_sem[g] = name
            rel_base[g] = 16 * ndma_b * j
            rel_after[g] = 16 * ndma_b * (j + 1)
    store_pos = {}
    inline_stores = set()
    for g in range(nchunk):
        pos = CHUNK * g + 34
        if pos < ntiles:
            store_pos.setdefault(pos, []).append(g)
            inline_stores.add(g)
    # SP tail: leftover stores ascending; the final chunk's relayout is
    # emitted one store earlier than strictly needed (it only gates on dve,
    # which resolves before the preceding store's gather does)
    tail_stores = [g for g in range(nchunk) if g not in inline_stores]
    act_stores = tail_stores[-2:] if len(tail_stores) >= 2 else []
    tail = []
    emitted = set(pool_chunks)
    for g in tail_stores:
        if g in act_stores:
            continue
        tail.append(("store", g))
    for g in sp_chunks:
        if g not in emitted:
            tail.append(("relayout", g))
            emitted.add(g)

    nc = bacc.Bacc()
    zhl_d = nc.dram_tensor("zhl", [2, 2, 128, ntiles * 128], bf16, kind="ExternalInput")
    nd3_d = nc.dram_tensor("nd3", [3, N], bf16, kind="ExternalInput")
    dhl_d = nc.dram_tensor("dhl", [2, 2, 128, N], bf16, kind="ExternalInput")
    nb_d = nc.dram_tensor("nb", [128, N], f32, kind="ExternalInput")
    dic_d = nc.dram_tensor("dic", [N, D], f32, kind="ExternalInput")
    e_d = nc.dram_tensor("e", [ntiles * 128, D], f32, kind="ExternalOutput")

    npair = (ntiles + 1) // 2

    ctx = ExitStack()
    with ctx:
        def sb(name, shape, dt):
            return ctx.enter_context(nc.sbuf_tensor(name, list(shape), dt))

        zhl = [sb(f"zhl{p}", (128, 2, 2, 256), bf16) for p in range(NZBUF)]
        wu = sb("warmup", (128, 512), bf16)
        dhl = sb("dhl_sb", (128, 2, 2, N), bf16)
        nb = sb("nb_sb", (128, N), f32)
        nd3_sb = sb("nd3_sb", (3, N), bf16)
        ones3 = sb("ones3", (3, 128), bf16)
        scores = [sb(f"scores{p}", (128, N), f32) for p in range(4)]
        m8 = [sb(f"m8_{p}", (128, 8), f32) for p in range(2)]
        staging = sb("staging", (128, ntiles, 8), u16)
        idxs2 = [sb(f"idxs{q}", (128, CHUNK, 8), i16) for q in range(2)]
        gth = [sb(f"gth{q}", (128, CHUNK, D), f32) for q in range(2)]
        ps = [ctx.enter_context(nc.psum_tensor(f"ps{j}", [128, N], f32))
              for j in range(4)]

        sem = {}
        for s in ("prepd", "prepdl", "prepn", "prep3", "m1", "z", "actba",
                  "actbb", "pem", "acts", "dve", "relp", "rels", "gth",
                  "outs", "outa"):
            sem[s] = ctx.enter_context(nc.semaphore(s))

        def relayout(eng, g):
            s, e = CHUNK * g, CHUNK * (g + 1)
            q = g % 2
            rs = sem[rel_sem[g]]
            eng.wait_ge(sem["dve"], e)
            if g >= 2:
                eng.wait_ge(sem["gth"], 16 * (g - 1))
            with nc.allow_non_contiguous_dma(reason="16x2B idx relayout"):
                for kk in range(8):
                    eng.dma_start(
                        out=idxs2[q][0:16, :, kk:kk + 1],
                        in_=staging[16 * kk:16 * (kk + 1), s:e, 0:1].bitcast(i16),
                    ).then_inc(rs, 16)
            if REPLICATE:
                eng.wait_ge(rs, rel_base[g] + 16 * 8)
                for r in range(1, 8):
                    eng.dma_start(
                        out=idxs2[q][16 * r:16 * (r + 1), :, :],
                        in_=idxs2[q][0:16, :, :],
                    ).then_inc(rs, 16)

        def store(eng, g, outsem="outs"):
            eng.wait_ge(sem["gth"], 16 * (g + 1))
            eng.dma_start(
                out=e_d[CHUNK * 128 * g:CHUNK * 128 * (g + 1), :].rearrange(
                    "(c p) d -> p c d", p=128),
                in_=gth[g % 2][:],
            ).then_inc(sem[outsem], 16)

        with nc.Block() as block:

            @block.sync
            def _(sync):
                sync.dma_start(
                    out=zhl[0][:],
                    in_=zhl_d[:, :, :, 0:256].rearrange("s c p t -> p s c t"),
                ).then_inc(sem["z"], 16)
                sync.dma_start(out=dhl[:, 0, :, :],
                               in_=dhl_d[0].rearrange("c p n -> p c n")
                               ).then_inc(sem["prepd"], 16)
                sync.dma_start(out=nd3_sb[:], in_=nd3_d[:]).then_inc(sem["prep3"], 16)
                sync.dma_start(out=nb[:], in_=nb_d[:]).then_inc(sem["prepn"], 16)
                sync.dma_start(out=dhl[:, 1, :, :],
                               in_=dhl_d[1].rearrange("c p n -> p c n")
                               ).then_inc(sem["prepdl"], 16)
                for j in range(1, npair):
                    i = 2 * j
                    if i >= 2 * NZBUF:
                        sync.wait_ge(sem["pem"], 2 * (i - 2 * NZBUF) + 4)
                    sync.dma_start(
                        out=zhl[j % NZBUF][:],
                        in_=zhl_d[:, :, :, i * 128:(i + 2) * 128].rearrange(
                            "s c p t -> p s c t"),
                    ).then_inc(sem["z"], 16)
                    for g in store_pos.get(i, []) + store_pos.get(i + 1, []):
                        store(sync, g)
                for kind, v in tail:
                    if kind == "relayout":
                        relayout(sync, v)
                    else:
                        store(sync, v)
                sync.wait_ge(sem["outs"], 16 * (nchunk - len(act_stores)))
                if act_stores:
                    n_outa = sum(2 if g == nchunk - 1 else 1 for g in act_stores)
                    sync.wait_ge(sem["outa"], 16 * n_outa)

            @block.scalar
            def _(scalar):
                scalar.wait_ge(sem["prepn"], 16)
                for i in range(ntiles):
                    p = i % 4
                    if i >= 4:
                        scalar.wait_ge(sem["dve"], i - 3)
                    scalar.wait_ge(sem["pem"], 2 * i + 1)
                    scalar.copy(scores[p][:, 0:512], ps[p][:, 0:512])
                    if i + 4 < ntiles:
                        scalar.drain()
                        scalar.copy(ps[p][:, 0:512],
                                    nb[:, 0:512]).then_inc(sem["actba"], 1)
                    scalar.wait_ge(sem["pem"], 2 * i + 2)
                    scalar.copy(scores[p][:, 512:1024],
                                ps[p][:, 512:1024]).then_inc(sem["acts"], 1)
                    if i + 4 < ntiles:
                        scalar.drain()
                        scalar.copy(ps[p][:, 512:1024],
                                    nb[:, 512:1024]).then_inc(sem["actbb"], 1)
                for g in act_stores:
                    if g == nchunk - 1:
                        for half in range(2):
                            scalar.wait_ge(sem["gth"], 16 * (g + 1 + half))
                            r0 = CHUNK * 128 * g + 512 * half
                            scalar.dma_start(
                                out=e_d[r0:r0 + 512, :].rearrange(
                                    "(c p) d -> p c d", p=128),
                                in_=gth[g % 2][:, 4 * half:4 * half + 4, :],
                            ).then_inc(sem["outa"], 16)
                    else:
                        store(scalar, g, outsem="outa")

            @block.tensor
            def _(tensor):
                # pstate warmup: dummy matmuls on garbage while input DMAs
                # land, so the PE clock is at full speed for the real stream
                for _ in range(10):
                    tensor.matmul(ps[0][:, 0:512], wu[:, 0:128], wu[:, 0:512],
                                  start=True, stop=True, skip_group_check=True)
                tensor.wait_ge(sem["prepd"], 16)
                tensor.wait_ge(sem["prep3"], 16)
                tensor.wait_ge(sem["m1"], 1)
                for i in range(ntiles):
                    p = i % 4
                    q = (i // 2) % NZBUF
                    tsl = slice(128 * (i % 2), 128 * (i % 2) + 128)
                    tensor.wait_ge(sem["z"], 16 * (i // 2 + 1))
                    for h in range(2):
                        if i >= 4:
                            tensor.wait_ge(sem["actba" if h == 0 else "actbb"],
                                           i - 3)
                        ns = bass.ts(h, 512)
                        mm = None
                        if i < 4:
                            tensor.matmul(ps[p][:, ns], ones3[:], nd3_sb[:, ns],
                                          start=True, stop=False,
                                          skip_group_check=True)
                        for t, (sz, sd) in enumerate(TERMS):
                            if i == 0 and h == 0 and sd == 1:
                                tensor.wait_ge(sem["prepdl"], 16)
                            for c in range(2):
                                mm = tensor.matmul(ps[p][:, ns],
                                                   zhl[q][:, sz, c, tsl],
                                                   dhl[:, sd, c, ns],
                                                   start=False,
                                                   stop=(t == 2 and c == 1),
                                                   skip_group_check=True)
                        mm.then_inc(sem["pem"], 1)

            @block.vector
            def _(vector):
                vector.memset(ones3[:], 1.0)
                vector.drain()
                vector.nop().then_inc(sem["m1"], 1)
                for i in range(ntiles):
                    p = i % 4
                    vector.wait_ge(sem["acts"], i + 1)
                    vector.max(m8[i % 2][:], scores[p][:])
                    vector.drain()
                    vector.max_index(staging[:, i, :], m8[i % 2][:],
                                     scores[p][:]).then_inc(sem["dve"], 1)

            @block.gpsimd
            def _(gpsimd):
                reg = gpsimd.to_reg(CHUNK * 128)
                for g in range(nchunk):
                    if g in pool_chunks:
                        relayout(gpsimd, g)
                    if g != nchunk - 1:
                        gpsimd.wait_ge(sem[rel_sem[g]], rel_after[g])
                    if g >= 2:
                        gpsimd.wait_ge(sem["outs"], 16 * (g - 1))
                    if g >= 1:
                        gpsimd.wait_ge(sem["gth"], 16 * g)
                    if g == nchunk - 1:
                        gpsimd.wait_ge(sem[rel_sem[g]], rel_after[g])
                        reg2 = gpsimd.to_reg(CHUNK * 64)
                        for half in range(2):
                            gpsimd.dma_gather(
                                out_ap=gth[g % 2][:, 4 * half:4 * half + 4, :],
                                in_ap=dic_d[:],
                                idxs_ap=idxs2[g % 2][:, 4 * half:4 * half + 4, :],
                                num_idxs=CHUNK * 64,
                                num_idxs_reg=reg2,
                                elem_size=D,
                                elem_step=D,
                            ).then_inc(sem["gth"], 16)
                    else:
                        gpsimd.dma_gather(
                            out_ap=gth[g % 2][:],
                            in_ap=dic_d[:],
                            idxs_ap=idxs2[g % 2][:],
                            num_idxs=CHUNK * 128,
                            num_idxs_reg=reg,
                            elem_size=D,
                            elem_step=D,
                        ).then_inc(sem["gth"], 16)

    nc.finalize()
    return nc


def _prep_dict(dictionary):
    dic = np.ascontiguousarray(dictionary.astype(np.float32))
    dT2 = np.ascontiguousarray(2.0 * dic.T).astype(np.float32)   # [D, N]
    dh = dT2.astype(ml_dtypes.bfloat16)
    dl = (dT2 - dh.astype(np.float32)).astype(ml_dtypes.bfloat16)
    dhl = np.ascontiguousarray(
        np.stack([dh.reshape(2, 128, N), dl.reshape(2, 128, N)]))
    nd = -(dic.astype(np.float64) ** 2).sum(-1)
    nb = np.ascontiguousarray(
        np.broadcast_to(nd.astype(np.float32), (128, N)))
    h1 = nd.astype(ml_dtypes.bfloat16)
    r1 = nd - h1.astype(np.float64)
    h2 = r1.astype(ml_dtypes.bfloat16)
    r2 = r1 - h2.astype(np.float64)
    h3 = r2.astype(ml_dtypes.bfloat16)
    nd3 = np.ascontiguousarray(np.stack([h1, h2, h3]).astype(ml_dtypes.bfloat16))
    return dic, dhl, nb, nd3


def _prep_ze(ze_core):
    zh = ze_core.astype(ml_dtypes.bfloat16)
    zl = (ze_core - zh.astype(np.float32)).astype(ml_dtypes.bfloat16)
    rows = ze_core.shape[0]
    return np.ascontiguousarray(
        np.stack([np.ascontiguousarray(zh.T).reshape(2, 128, rows),
                  np.ascontiguousarray(zl.T).reshape(2, 128, rows)]))


def kernel(ze, dictionary):
    if "nc" not in _CACHE:
        _CACHE["nc"] = build()
    nc = _CACHE["nc"]
    dic, dhl, nb, nd3 = _prep_dict(np.asarray(dictionary))
    ze = np.ascontiguousarray(np.asarray(ze, dtype=np.float32))
    zec = ze.reshape(CORES, ROWS, D)
    in_maps = [{"zhl": _prep_ze(zec[c]), "dhl": dhl, "nb": nb, "nd3": nd3,
                "dic": dic} for c in range(CORES)]
    res = run_bass_kernel_spmd(nc, in_maps, list(range(CORES)))
    e = np.stack([res.results[c]["e"] for c in range(CORES)])
    return e.reshape(B, T, D)
